# revision 8
# baseline (speedup 1.0000x reference)
"""Sparse-attention (sliding window 512 + front 256) Trainium2 kernel.

Sequence-sharded across 8 NeuronCores: core c owns queries [512c, 512c+512)
and computes ALL 16 heads for them, including the full output projection, so
per-core output is a disjoint y slice [512, 2048] (no cross-core reduction).

Layout: scores are computed TRANSPOSED (sT[k, q] = kT_tile^T @ qT), so the
exp() output is already the [keys, q] rhs operand the attn@V matmul needs --
no PE transposes and no PSUM->SBUF staging copies.

v2 restructure (from trace analysis of v1 at 310us):
  - fp16 instead of bf16 everywhere (same PE rate, 4 extra mantissa bits).
  - q projection is interleaved INTO the attention head loop, so the PE
    stays saturated through the exp-heavy window (v1's phase B was
    Scalar-walled at ~105% busy while the PE idled).
  - softmax key-sum no longer runs on the PE (v1: 10 row-select matmuls per
    head, 57k cycles): the exp tiles accumulate on the DVE (fp16 2x mode)
    and a single ones-column matmul per head does the 128-partition reduce.
  - dead packed-key rows (core 0's padded band + covered front) are killed
    with a per-partition bias of -60 inside the exp activation itself
    (exp(s/sqrt(d) - 60) ~ 0), replacing the v1 row-select vector.

Per-core data variation (uniform single program):
  - packed key layout (NT=10 tiles): cols 0:256 front, cols 256:1280 band
    positions [qlo-512, qlo+512), zero-x for padding;
  - 11 sparse 128x128 {0,1} mask blocks multiplied into exp(scores);
  - biasv [128, NT]: -60 on core 0's dead tiles, 0 elsewhere.
"""
import math
import sys

import numpy as np

sys.path.insert(0, "/opt/trn_rl_repo")

import concourse.bass as bass
from concourse import bacc
import concourse.mybir as mybir
import concourse.tile as tile
from concourse.bass_utils import run_bass_kernel_spmd

# Problem constants (hardcoded per contract)
S = 4096
D = 2048
NH = 16
NKV = 4
NREP = NH // NKV
DQK = 128
DV = 128
WIN = 512
FRONT = 256
THETA = 10000.0
P = 128
NC_ = 8          # cores
SC = S // NC_    # 512 queries per core
KO = D // P      # 16 contraction chunks
NT = 10          # packed key tiles per core (2 front + 8 band)
NKC = NT * P     # 1280 packed key positions
NQTL = 4         # query tiles per core

F32 = mybir.dt.float32
FP16 = mybir.dt.float16


def build_program():
    nc = bacc.Bacc(None, target_bir_lowering=False)

    x_d = nc.dram_tensor("xp", [P, KO, NKC], FP16, kind="ExternalInput")
    wq_d = nc.dram_tensor("wq", [P, 4, KO, 4 * DQK], FP16, kind="ExternalInput")
    wk_d = nc.dram_tensor("wk", [P, NKV, KO, DQK], FP16, kind="ExternalInput")
    wv_d = nc.dram_tensor("wv", [P, KO, NKV * DV], FP16, kind="ExternalInput")
    wo_d = nc.dram_tensor("wo", [P, 4, NH, SC], FP16, kind="ExternalInput")
    cos_d = nc.dram_tensor("cosd", [P, NKC], FP16, kind="ExternalInput")
    sin_d = nc.dram_tensor("sind", [P, NKC], FP16, kind="ExternalInput")
    b_d = nc.dram_tensor("bmask", [P, 11, P], FP16, kind="ExternalInput")
    bias_d = nc.dram_tensor("biasv", [P, NT], F32, kind="ExternalInput")
    y_d = nc.dram_tensor("y", [SC, D], FP16, kind="ExternalOutput")

    inv_sqrt_dqk = 1.0 / math.sqrt(DQK)
    qc0 = NKC - SC  # first packed col of this core's own queries (768)

    # Per key tile t, the q columns that can attend it:
    #   front tiles (t=0,1): all 512;  band tile b: qtl in [b-4, b].
    qr = {0: (0, SC), 1: (0, SC)}
    for b in range(8):
        lo = max(0, b - 4) * P
        hi = (min(3, b) + 1) * P
        qr[2 + b] = (lo, hi - lo)
    # masked 128-col blocks per tile: (block index, absolute col offset)
    mask_blocks = {0: [(0, 0)], 1: [(1, 0)]}
    for b in range(8):
        off = (b if b < 4 else b - 4) * P
        mask_blocks[2 + b] = [(3 + b, off)]

    with tile.TileContext(nc) as tc:
        with (
            tc.tile_pool(name="persist", bufs=1) as persist,
            tc.tile_pool(name="ps", bufs=4, space="PSUM") as ps,
            tc.tile_pool(name="psO", bufs=2, space="PSUM") as psO,
            tc.tile_pool(name="psL", bufs=2, space="PSUM") as psL,
        ):
            # ---- persistent SBUF ----
            qT = persist.tile([P, NH, SC], FP16, tag="qT")
            kT = persist.tile([P, NKV, NKC], FP16, tag="kT")
            v_sb = persist.tile([P, NT, NKV * DV], FP16, tag="v")
            outT = persist.tile([P, NH, SC], FP16, tag="outT")
            b_sb = persist.tile([P, 11, P], FP16, tag="bm")
            bias_sb = persist.tile([P, NT], F32, tag="biasv")
            ones_row = persist.tile([1, P], FP16, tag="onesr")
            ones_col = persist.tile([P, 1], FP16, tag="onesc")
            # own-query x slice and cos/sin live through phase B (q proj is
            # interleaved there); the full packed x is phase-A scoped
            xq_sb = persist.tile([P, KO, SC], FP16, tag="xq")
            cos_sb = persist.tile([P, NKC], FP16, tag="cos")
            sin_sb = persist.tile([P, NKC], FP16, tag="sin")

            nc.vector.memset(ones_row[:], 1.0)
            nc.vector.memset(ones_col[:], 1.0)

            def rope(dst, psrc, cosap, sinap, pool, w):
                """dst(fp16) = RoPE(psrc) in the paired [re(64); im(64)] basis.

                sw = [-im; re]; dst = psrc*cos + sw*sin.
                """
                sw = pool.tile([P, w], F32, tag="sw")
                nc.scalar.mul(sw[0:64], psrc[64:128], -1.0)
                nc.scalar.copy(sw[64:128], psrc[0:64])
                trc = pool.tile([P, w], F32, tag="trc")
                nc.vector.tensor_tensor(
                    trc[:], psrc, cosap, op=mybir.AluOpType.mult
                )
                nc.vector.tensor_tensor(
                    sw[:], sw[:], sinap, op=mybir.AluOpType.mult
                )
                nc.vector.tensor_tensor(
                    dst, trc[:], sw[:], op=mybir.AluOpType.add
                )

            # ---- Phase A: k/v projections + k RoPE ----
            with tc.tile_pool(name="phA", bufs=1) as pa, tc.tile_pool(
                name="ropep", bufs=3
            ) as rp:
                x_sb = pa.tile([P, KO, NKC], FP16, tag="x")
                wk_sb = pa.tile([P, NKV, KO, DQK], FP16, tag="wk")
                wv_sb = pa.tile([P, KO, NKV * DV], FP16, tag="wv")

                nc.sync.dma_start(wk_sb[:, 0, 0:4], wk_d[:, 0, 0:4])
                nc.sync.dma_start(x_sb[:, 0:2, 0:512], x_d[:, 0:2, 0:512])
                nc.sync.dma_start(x_sb[:, 2:4, 0:512], x_d[:, 2:4, 0:512])
                nc.sync.dma_start(wk_sb[:, 0, 4:16], wk_d[:, 0, 4:16])
                for kg in range(4, KO, 4):
                    nc.sync.dma_start(
                        x_sb[:, kg : kg + 4, 0:512],
                        x_d[:, kg : kg + 4, 0:512],
                    )
                nc.sync.dma_start(cos_sb[:, 0:512], cos_d[:, 0:512])
                nc.sync.dma_start(sin_sb[:, 0:512], sin_d[:, 0:512])
                for _kvh in range(1, NKV):
                    nc.sync.dma_start(wk_sb[:, _kvh], wk_d[:, _kvh])
                nc.sync.dma_start(wv_sb[:], wv_d[:])
                for c0 in range(512, NKC, 512):
                    cw = min(512, NKC - c0)
                    nc.sync.dma_start(
                        x_sb[:, :, c0 : c0 + cw], x_d[:, :, c0 : c0 + cw]
                    )
                    nc.sync.dma_start(
                        cos_sb[:, c0 : c0 + cw], cos_d[:, c0 : c0 + cw]
                    )
                    nc.sync.dma_start(
                        sin_sb[:, c0 : c0 + cw], sin_d[:, c0 : c0 + cw]
                    )
                nc.sync.dma_start(b_sb[:], b_d[:])
                nc.sync.dma_start(bias_sb[:], bias_d[:])
                # own-query x slice for the interleaved q projection
                nc.sync.dma_start(xq_sb[:], x_d[:, :, qc0:NKC])

                # k+v projection, chunk-outer so compute starts on chunk 0
                for c0 in range(0, NKC, 512):
                    cw = min(512, NKC - c0)
                    for kvh in range(NKV):
                        psk = ps.tile([P, SC], F32, tag="big", name="psk")
                        psk = psk[:, :cw]
                        for ko in range(KO):
                            nc.tensor.matmul(
                                psk,
                                wk_sb[:, kvh, ko, :],
                                x_sb[:, ko, c0 : c0 + cw],
                                start=(ko == 0),
                                stop=(ko == KO - 1),
                            )
                        rope(
                            kT[:, kvh, c0 : c0 + cw],
                            psk,
                            cos_sb[:, c0 : c0 + cw],
                            sin_sb[:, c0 : c0 + cw],
                            rp,
                            cw,
                        )
                    # v projection (natural [keys, dv], all 4 kv heads)
                    for t in range(c0 // P, min(NT, (c0 + cw) // P)):
                        psv = ps.tile([P, SC], F32, tag="big", name="psv")
                        psv = psv[:, : NKV * DV]
                        for ko in range(KO):
                            nc.tensor.matmul(
                                psv,
                                x_sb[:, ko, t * P : (t + 1) * P],
                                wv_sb[:, ko, :],
                                start=(ko == 0),
                                stop=(ko == KO - 1),
                            )
                        nc.vector.tensor_copy(v_sb[:, t, :], psv)

            # ---- Phase B: q proj + RoPE + attention, per head, interleaved --
            # phase C pools opened now so wo prefetch overlaps phase B
            pc = tc.alloc_tile_pool(name="phC", bufs=2)
            pcy = tc.alloc_tile_pool(name="phCy", bufs=4)
            with (
                tc.tile_pool(name="phB", bufs=3) as pb,
                tc.tile_pool(name="phBs", bufs=4) as pbs,
                tc.tile_pool(name="wqs", bufs=2) as wqs,
                tc.tile_pool(name="ropeq", bufs=3) as rq,
            ):
                tails = [None] * NH  # (psl, pso) pending normalize
                wq_gs = [None] * (NH // 2)

                def fetch_wq(g):
                    # 2-head chunk: [P, KO, 2*DQK] slice of the 4-head group
                    gg, hh = divmod(2 * g, 4)
                    wq_g = wqs.tile([P, KO, 2 * DQK], FP16, tag="wqg")
                    nc.sync.dma_start(
                        wq_g[:], wq_d[:, gg, :, hh * DQK : (hh + 2) * DQK]
                    )
                    wq_gs[g] = wq_g

                def emit_qproj(h):
                    g, hh = divmod(h, 2)
                    if hh == 0 and g + 1 < NH // 2:
                        fetch_wq(g + 1)
                    psq = ps.tile([P, SC], F32, tag="big", name="psq")
                    for ko in range(KO):
                        nc.tensor.matmul(
                            psq[:],
                            wq_gs[g][:, ko, hh * DQK : (hh + 1) * DQK],
                            xq_sb[:, ko, :],
                            start=(ko == 0),
                            stop=(ko == KO - 1),
                        )
                    rope(
                        qT[:, h, :],
                        psq[:],
                        cos_sb[:, qc0:NKC],
                        sin_sb[:, qc0:NKC],
                        rq,
                        SC,
                    )

                def emit_scores(h):
                    """scores + exp into one [P, NT, SC] tile; DVE-accumulate
                    the key-sum into acc; returns (pTh, acc)."""
                    kvh = h // NREP
                    pTh = pb.tile([P, NT, SC], FP16, tag="pT", name="pTh")
                    acc = pbs.tile([P, SC], FP16, tag="acc", name="acc")
                    for ti in range(NT):
                        q0, qw = qr[ti]
                        pst = ps.tile([P, SC], F32, tag="big", name="pst")
                        pst = pst[:, :qw]
                        nc.tensor.matmul(
                            pst,
                            kT[:, kvh, ti * P : (ti + 1) * P],
                            qT[:, h, q0 : q0 + qw],
                            start=True,
                            stop=True,
                        )
                        pTt = pTh[:, ti, q0 : q0 + qw]
                        nc.scalar.activation(
                            pTt,
                            pst,
                            mybir.ActivationFunctionType.Exp,
                            scale=inv_sqrt_dqk,
                            bias=bias_sb[:, ti : ti + 1],
                        )
                        for blk, off in mask_blocks[ti]:
                            bw = 2 * P if ti == 1 else P
                            nc.vector.tensor_tensor(
                                pTh[:, ti, off : off + bw],
                                pTh[:, ti, off : off + bw],
                                b_sb[:, blk : blk + bw // P, :],
                                op=mybir.AluOpType.mult,
                            )
                        # fold tile into the key-sum accumulator (DVE)
                        if ti == 1:
                            nc.vector.tensor_tensor(
                                acc[:],
                                pTh[:, 0, :],
                                pTh[:, 1, :],
                                op=mybir.AluOpType.add,
                            )
                        elif ti >= 2:
                            nc.vector.tensor_tensor(
                                acc[:, q0 : q0 + qw],
                                acc[:, q0 : q0 + qw],
                                pTt,
                                op=mybir.AluOpType.add,
                            )
                    return pTh, acc

                def emit_la(h, pTh, acc):
                    kvh = h // NREP
                    pso = psO.tile([P, SC], F32, tag="o", name="pso")
                    psl = psL.tile([1, SC], F32, tag="l", name="psl")
                    # 128-partition reduce of acc: single ones-column matmul
                    nc.tensor.matmul(
                        psl[:], ones_col[:], acc[:], start=True, stop=True
                    )
                    for ti in range(NT):
                        q0, qw = qr[ti]
                        nc.tensor.matmul(
                            pso[:, q0 : q0 + qw],
                            v_sb[:, ti, kvh * DV : (kvh + 1) * DV],
                            pTh[:, ti, q0 : q0 + qw],
                            start=(ti == 0),
                            stop=(ti == NT - 1),
                            skip_group_check=True,
                        )
                    tails[h] = (psl, pso)

                def tail_recip(h):
                    psl, _ = tails[h]
                    irl = pbs.tile([1, SC], F32, tag="irl", name="irl")
                    nc.vector.reciprocal_approx_fast(irl[:], psl[:])
                    lrow = pbs.tile([1, SC], FP16, tag="lrow", name="lrow")
                    nc.scalar.copy(lrow[:], irl[:])
                    return lrow

                def tail_apply(h, lrow):
                    _, pso = tails[h]
                    psbc = ps.tile([P, SC], F32, tag="big", name="psbc")
                    nc.tensor.matmul(
                        psbc[:], ones_row[:], lrow[:], start=True, stop=True
                    )
                    rlbc = pbs.tile([P, SC], FP16, tag="rlbc", name="rlbc")
                    nc.vector.tensor_copy(rlbc[:], psbc[:])
                    nc.vector.tensor_tensor(
                        outT[:, h, :], pso[:], rlbc[:], op=mybir.AluOpType.mult
                    )
                    tails[h] = None

                fetch_wq(0)
                emit_qproj(0)
                for h in range(NH):
                    lr = None
                    if h > 0:
                        lr = tail_recip(h - 1)
                    pTh, acc = emit_scores(h)
                    if h + 1 < NH:
                        emit_qproj(h + 1)
                    if h > 0:
                        tail_apply(h - 1, lr)
                    emit_la(h, pTh, acc)
                tail_apply(NH - 1, tail_recip(NH - 1))

            # ---- Phase C: y = outT^T @ wo (stream wo in n-chunks) ----
            y_tiles = [
                pcy.tile([P, D], FP16, tag="y", name=f"y{i}")
                for i in range(NQTL)
            ]
            for ncl in range(4):
                wo_g = pc.tile([P, NH, SC], FP16, tag="wog", name="wo_g")
                nc.sync.dma_start(wo_g[:], wo_d[:, ncl])
                for qtl in range(NQTL):
                    psy = ps.tile([P, SC], F32, tag="big", name="psy")
                    for h in range(NH):
                        nc.tensor.matmul(
                            psy[:],
                            outT[:, h, qtl * P : (qtl + 1) * P],
                            wo_g[:, h, :],
                            start=(h == 0),
                            stop=(h == NH - 1),
                        )
                    nc.vector.tensor_copy(
                        y_tiles[qtl][:, ncl * SC : (ncl + 1) * SC], psy[:]
                    )
                    nc.sync.dma_start(
                        y_d[
                            qtl * P : (qtl + 1) * P,
                            ncl * SC : (ncl + 1) * SC,
                        ],
                        y_tiles[qtl][:, ncl * SC : (ncl + 1) * SC],
                    )
            pcy.release()
            pc.release()

    return nc


_PROGRAM = None


def _get_program():
    global _PROGRAM
    if _PROGRAM is None:
        _PROGRAM = build_program()
        _PROGRAM.finalize()
    return _PROGRAM


def _host_inputs(x, wq, wk, wv, wo):
    """Per-core input packing (all arrays contiguous, uniform shapes)."""
    F16 = np.float16
    x2 = np.asarray(x, np.float32).reshape(S, D)
    xT = np.ascontiguousarray(x2.T)  # [D, S]
    xr = xT.reshape(KO, P, S)  # [ko, p, s]

    # paired RoPE basis permutation within each head
    perm = np.concatenate([np.arange(0, DQK, 2), np.arange(1, DQK, 2)])
    wq_p = np.asarray(wq, np.float32).reshape(D, NH, DQK)[:, :, perm]
    wk_p = np.asarray(wk, np.float32).reshape(D, NKV, DQK)[:, :, perm]
    wv_r = np.asarray(wv, np.float32).reshape(D, NKV * DV)
    wo_r = np.asarray(wo, np.float32).reshape(NH, DV, D)

    # device layouts independent of core
    wq_dev = np.ascontiguousarray(
        wq_p.reshape(KO, P, NH, DQK)  # [ko, p, h, dqk]
        .reshape(KO, P, 4, 4 * DQK)  # group 4 heads
        .transpose(1, 2, 0, 3)  # [p, g, ko, 4*dqk]
    ).astype(F16)
    wk_dev = np.ascontiguousarray(
        wk_p.reshape(KO, P, NKV, DQK).transpose(1, 2, 0, 3)
    ).astype(F16)
    wv_dev = np.ascontiguousarray(
        wv_r.reshape(KO, P, NKV * DV).transpose(1, 0, 2)
    ).astype(F16)
    wo_dev = np.ascontiguousarray(
        wo_r.reshape(NH, DV, 4, SC).transpose(1, 2, 0, 3)  # [dv, ncl, h, sc]
    ).astype(F16)

    inv_freq = 1.0 / (THETA ** (np.arange(0, DQK, 2)[: DQK // 2] / DQK))

    in_maps = []
    for c in range(NC_):
        qlo = c * SC
        band_lo = qlo - WIN
        # packed key positions; garbage (pos<0) -> position 0, zero x
        pos = np.empty(NKC, np.int64)
        pos[:FRONT] = np.arange(FRONT)
        pos[FRONT:] = band_lo + np.arange(NKC - FRONT)
        valid = pos >= 0
        pos_c = np.where(valid, pos, 0)

        xp = xr[:, :, pos_c] * valid[None, None, :]  # [ko, p, nkc]
        if c == 0:
            # front tiles are dead on core 0 (band covers them); zero x so
            # their v projection is 0
            xp[:, :, :FRONT] = 0.0
        xp = np.ascontiguousarray(xp.transpose(1, 0, 2)).astype(F16)

        ang = np.outer(pos_c.astype(np.float64), inv_freq)  # (nkc, 64)
        cos_h = np.cos(ang).T.astype(np.float32)  # (64, nkc)
        sin_h = np.sin(ang).T.astype(np.float32)
        cos_p = np.ascontiguousarray(np.concatenate([cos_h, cos_h], 0)).astype(
            F16
        )
        sin_p = np.ascontiguousarray(np.concatenate([sin_h, sin_h], 0)).astype(
            F16
        )

        # Sparse 128x128 mask blocks (only blocks non-full on some core):
        #   blk 0: (front0, qtl0)  blk 1: (front1, qtl0)  blk 2: (front1, qtl1)
        #   blk 3+b: (band b, qtl b) for b<4 else (band b, qtl b-4)
        # Front tiles use only front_ok, band tiles only the band condition.
        r = np.arange(P)[:, None]
        B = np.zeros((P, 11, P), np.float32)
        blocks = [(0, 0), (1, 0), (1, 1)] + [
            (2 + b, b if b < 4 else b - 4) for b in range(8)
        ]
        for blk, (t, qtl) in enumerate(blocks):
            qpos = qlo + qtl * P + np.arange(P)[None, :]
            if t < 2:
                kpos = t * P + r
                allowed = (kpos < FRONT) & (kpos <= qpos - WIN)
            else:
                kpos = band_lo + (t - 2) * P + r
                allowed = (kpos >= 0) & (kpos <= qpos) & (kpos > qpos - WIN)
            B[:, blk, :] = allowed
        Bp = np.ascontiguousarray(B).astype(F16)

        # exp bias: kill core 0's dead tiles (front 0,1 + band 2..5 whose
        # packed positions are all negative) so they contribute ~0 to the
        # key-sum; all other (core, tile) pairs are fully live or handled
        # by the mask blocks.
        biasv = np.zeros((P, NT), np.float32)
        if c == 0:
            biasv[:, :6] = -60.0
        in_maps.append(
            {
                "xp": xp,
                "wq": wq_dev,
                "wk": wk_dev,
                "wv": wv_dev,
                "wo": wo_dev,
                "cosd": cos_p,
                "sind": sin_p,
                "bmask": Bp,
                "biasv": biasv,
            }
        )
    return in_maps


def kernel(x, wq, wk, wv, wo, _trace=False, _trace_kwargs=None):
    nc = _get_program()
    in_maps = _host_inputs(x, wq, wk, wv, wo)
    res = run_bass_kernel_spmd(
        nc, in_maps, list(range(NC_)), trace=_trace, **(_trace_kwargs or {})
    )
    y = np.concatenate(
        [np.asarray(r["y"], np.float32) for r in res.results], axis=0
    )
    out = y.reshape(1, S, D)
    if _trace:
        return out, res
    return out


# revision 9
# speedup vs baseline: 1.1620x; 1.1620x over previous
"""Sparse-attention (sliding window 512 + front 256) Trainium2 kernel.

Sequence-sharded across 8 NeuronCores: core c owns queries [512c, 512c+512)
and computes ALL 16 heads for them, including the full output projection, so
per-core output is a disjoint y slice [512, 2048] (no cross-core reduction).

Layout: scores are computed TRANSPOSED (sT[k, q] = kT_tile^T @ qT), so the
exp() output is already the [keys, q] rhs operand the attn@V matmul needs --
no PE transposes and no PSUM->SBUF staging copies.

v2 restructure (from trace analysis of v1 at 310us):
  - fp16 instead of bf16 everywhere (same PE rate, 4 extra mantissa bits).
  - q projection is interleaved INTO the attention head loop, so the PE
    stays saturated through the exp-heavy window (v1's phase B was
    Scalar-walled at ~105% busy while the PE idled).
  - softmax key-sum no longer runs on the PE (v1: 10 row-select matmuls per
    head, 57k cycles): the exp tiles accumulate on the DVE (fp16 2x mode)
    and a single ones-column matmul per head does the 128-partition reduce.
  - dead packed-key rows (core 0's padded band + covered front) are killed
    with a per-partition bias of -60 inside the exp activation itself
    (exp(s/sqrt(d) - 60) ~ 0), replacing the v1 row-select vector.

Per-core data variation (uniform single program):
  - packed key layout (NT=10 tiles): cols 0:256 front, cols 256:1280 band
    positions [qlo-512, qlo+512), zero-x for padding;
  - 11 sparse 128x128 {0,1} mask blocks multiplied into exp(scores);
  - biasv [128, NT]: -60 on core 0's dead tiles, 0 elsewhere.
"""
import math
import sys

import numpy as np

sys.path.insert(0, "/opt/trn_rl_repo")

import concourse.bass as bass
from concourse import bacc
import concourse.mybir as mybir
import concourse.tile as tile
from concourse.bass_utils import run_bass_kernel_spmd

# Problem constants (hardcoded per contract)
S = 4096
D = 2048
NH = 16
NKV = 4
NREP = NH // NKV
DQK = 128
DV = 128
WIN = 512
FRONT = 256
THETA = 10000.0
P = 128
NC_ = 8          # cores
SC = S // NC_    # 512 queries per core
KO = D // P      # 16 contraction chunks
NT = 10          # packed key tiles per core (2 front + 8 band)
NKC = NT * P     # 1280 packed key positions
NQTL = 4         # query tiles per core

F32 = mybir.dt.float32
FP16 = mybir.dt.float16


def build_program():
    nc = bacc.Bacc(None, target_bir_lowering=False)

    x_d = nc.dram_tensor("xp", [P, KO, NKC], FP16, kind="ExternalInput")
    wq_d = nc.dram_tensor("wq", [P, 4, KO, 4 * DQK], FP16, kind="ExternalInput")
    wk_d = nc.dram_tensor("wk", [P, NKV, KO, DQK], FP16, kind="ExternalInput")
    wv_d = nc.dram_tensor("wv", [P, KO, NKV * DV], FP16, kind="ExternalInput")
    wo_d = nc.dram_tensor("wo", [P, 4, NH, SC], FP16, kind="ExternalInput")
    cos_d = nc.dram_tensor("cosd", [P, NKC], FP16, kind="ExternalInput")
    sin_d = nc.dram_tensor("sind", [P, NKC], FP16, kind="ExternalInput")
    b_d = nc.dram_tensor("bmask", [P, 11, P], FP16, kind="ExternalInput")
    bias_d = nc.dram_tensor("biasv", [P, NT], F32, kind="ExternalInput")
    y_d = nc.dram_tensor("y", [SC, D], FP16, kind="ExternalOutput")

    inv_sqrt_dqk = 1.0 / math.sqrt(DQK)
    qc0 = NKC - SC  # first packed col of this core's own queries (768)

    # Per key tile t, the q columns that can attend it:
    #   front tiles (t=0,1): all 512;  band tile b: qtl in [b-4, b].
    qr = {0: (0, SC), 1: (0, SC)}
    for b in range(8):
        lo = max(0, b - 4) * P
        hi = (min(3, b) + 1) * P
        qr[2 + b] = (lo, hi - lo)
    # masked 128-col blocks per tile: (block index, absolute col offset)
    mask_blocks = {0: [(0, 0)], 1: [(1, 0)]}
    for b in range(8):
        off = (b if b < 4 else b - 4) * P
        mask_blocks[2 + b] = [(3 + b, off)]

    with tile.TileContext(nc) as tc:
        with (
            tc.tile_pool(name="persist", bufs=1) as persist,
            tc.tile_pool(name="ps", bufs=4, space="PSUM") as ps,
            tc.tile_pool(name="psO", bufs=2, space="PSUM") as psO,
            tc.tile_pool(name="psL", bufs=2, space="PSUM") as psL,
        ):
            # ---- persistent SBUF ----
            qT = persist.tile([P, NH, SC], FP16, tag="qT")
            kT = persist.tile([P, NKV, NKC], FP16, tag="kT")
            v_sb = persist.tile([P, NT, NKV * DV], FP16, tag="v")
            outT = persist.tile([P, NH, SC], FP16, tag="outT")
            b_sb = persist.tile([P, 11, P], FP16, tag="bm")
            bias_sb = persist.tile([P, NT], F32, tag="biasv")
            ones_row = persist.tile([1, P], FP16, tag="onesr")
            ones_col = persist.tile([P, 1], FP16, tag="onesc")
            # own-query x slice and cos/sin live through phase B (q proj is
            # interleaved there); the full packed x is phase-A scoped
            xq_sb = persist.tile([P, KO, SC], FP16, tag="xq")
            cos_sb = persist.tile([P, NKC], FP16, tag="cos")
            sin_sb = persist.tile([P, NKC], FP16, tag="sin")

            nc.vector.memset(ones_row[:], 1.0)
            nc.vector.memset(ones_col[:], 1.0)

            def rope(dst, psrc, cosap, sinap, pool, w):
                """dst(fp16) = RoPE(psrc) in the paired [re(64); im(64)] basis.

                sw = [-im; re]; dst = psrc*cos + sw*sin.
                """
                sw = pool.tile([P, w], F32, tag="sw")
                nc.scalar.mul(sw[0:64], psrc[64:128], -1.0)
                nc.scalar.copy(sw[64:128], psrc[0:64])
                trc = pool.tile([P, w], F32, tag="trc")
                nc.vector.tensor_tensor(
                    trc[:], psrc, cosap, op=mybir.AluOpType.mult
                )
                nc.vector.tensor_tensor(
                    sw[:], sw[:], sinap, op=mybir.AluOpType.mult
                )
                nc.vector.tensor_tensor(
                    dst, trc[:], sw[:], op=mybir.AluOpType.add
                )

            # ---- Phase A: k/v projections + k RoPE ----
            with tc.tile_pool(name="phA", bufs=1) as pa, tc.tile_pool(
                name="ropep", bufs=3
            ) as rp:
                x_sb = pa.tile([P, KO, NKC], FP16, tag="x")
                wk_sb = pa.tile([P, NKV, KO, DQK], FP16, tag="wk")
                wv_sb = pa.tile([P, KO, NKV * DV], FP16, tag="wv")

                nc.sync.dma_start(wk_sb[:, 0, 0:4], wk_d[:, 0, 0:4])
                nc.sync.dma_start(x_sb[:, 0:2, 0:512], x_d[:, 0:2, 0:512])
                nc.sync.dma_start(x_sb[:, 2:4, 0:512], x_d[:, 2:4, 0:512])
                nc.sync.dma_start(wk_sb[:, 0, 4:16], wk_d[:, 0, 4:16])
                for kg in range(4, KO, 4):
                    nc.sync.dma_start(
                        x_sb[:, kg : kg + 4, 0:512],
                        x_d[:, kg : kg + 4, 0:512],
                    )
                nc.sync.dma_start(cos_sb[:, 0:512], cos_d[:, 0:512])
                nc.sync.dma_start(sin_sb[:, 0:512], sin_d[:, 0:512])
                for _kvh in range(1, NKV):
                    nc.sync.dma_start(wk_sb[:, _kvh], wk_d[:, _kvh])
                nc.sync.dma_start(wv_sb[:], wv_d[:])
                for c0 in range(512, NKC, 512):
                    cw = min(512, NKC - c0)
                    nc.sync.dma_start(
                        x_sb[:, :, c0 : c0 + cw], x_d[:, :, c0 : c0 + cw]
                    )
                    nc.sync.dma_start(
                        cos_sb[:, c0 : c0 + cw], cos_d[:, c0 : c0 + cw]
                    )
                    nc.sync.dma_start(
                        sin_sb[:, c0 : c0 + cw], sin_d[:, c0 : c0 + cw]
                    )
                nc.sync.dma_start(b_sb[:], b_d[:])
                nc.sync.dma_start(bias_sb[:], bias_d[:])
                # own-query x slice for the interleaved q projection
                nc.sync.dma_start(xq_sb[:], x_d[:, :, qc0:NKC])

                # k+v projection, chunk-outer so compute starts on chunk 0
                for c0 in range(0, NKC, 512):
                    cw = min(512, NKC - c0)
                    for kvh in range(NKV):
                        psk = ps.tile([P, SC], F32, tag="big", name="psk")
                        psk = psk[:, :cw]
                        for ko in range(KO):
                            nc.tensor.matmul(
                                psk,
                                wk_sb[:, kvh, ko, :],
                                x_sb[:, ko, c0 : c0 + cw],
                                start=(ko == 0),
                                stop=(ko == KO - 1),
                            )
                        rope(
                            kT[:, kvh, c0 : c0 + cw],
                            psk,
                            cos_sb[:, c0 : c0 + cw],
                            sin_sb[:, c0 : c0 + cw],
                            rp,
                            cw,
                        )
                    # v projection (natural [keys, dv], all 4 kv heads)
                    for t in range(c0 // P, min(NT, (c0 + cw) // P)):
                        psv = ps.tile([P, SC], F32, tag="big", name="psv")
                        psv = psv[:, : NKV * DV]
                        for ko in range(KO):
                            nc.tensor.matmul(
                                psv,
                                x_sb[:, ko, t * P : (t + 1) * P],
                                wv_sb[:, ko, :],
                                start=(ko == 0),
                                stop=(ko == KO - 1),
                            )
                        nc.vector.tensor_copy(v_sb[:, t, :], psv)

            # ---- Phase B: q proj + RoPE + attention, per head, interleaved --
            # phase C pools opened now so wo prefetch overlaps phase B
            pc = tc.alloc_tile_pool(name="phC", bufs=2)
            pcy = tc.alloc_tile_pool(name="phCy", bufs=4)
            with (
                tc.tile_pool(name="phB", bufs=3) as pb,
                tc.tile_pool(name="phBs", bufs=4) as pbs,
                tc.tile_pool(name="wqs", bufs=2) as wqs,
                tc.tile_pool(name="ropeq", bufs=3) as rq,
            ):
                tails = [None] * NH  # (psl, pso) pending normalize
                wq_gs = [None] * (NH // 2)

                def fetch_wq(g):
                    # 2-head chunk: [P, KO, 2*DQK] slice of the 4-head group
                    gg, hh = divmod(2 * g, 4)
                    wq_g = wqs.tile([P, KO, 2 * DQK], FP16, tag="wqg")
                    nc.sync.dma_start(
                        wq_g[:], wq_d[:, gg, :, hh * DQK : (hh + 2) * DQK]
                    )
                    wq_gs[g] = wq_g

                def emit_qproj(h):
                    g, hh = divmod(h, 2)
                    if hh == 0 and g + 1 < NH // 2:
                        fetch_wq(g + 1)
                    psq = ps.tile([P, SC], F32, tag="big", name="psq")
                    for ko in range(KO):
                        nc.tensor.matmul(
                            psq[:],
                            wq_gs[g][:, ko, hh * DQK : (hh + 1) * DQK],
                            xq_sb[:, ko, :],
                            start=(ko == 0),
                            stop=(ko == KO - 1),
                        )
                    rope(
                        qT[:, h, :],
                        psq[:],
                        cos_sb[:, qc0:NKC],
                        sin_sb[:, qc0:NKC],
                        rq,
                        SC,
                    )

                def emit_scores(h):
                    """scores + exp into one [P, NT, SC] tile; DVE-accumulate
                    the key-sum into acc; returns (pTh, acc)."""
                    kvh = h // NREP
                    pTh = pb.tile([P, NT, SC], FP16, tag="pT", name="pTh")
                    acc = pbs.tile([P, SC], FP16, tag="acc", name="acc")
                    for ti in range(NT):
                        q0, qw = qr[ti]
                        pst = ps.tile([P, SC], F32, tag="big", name="pst")
                        pst = pst[:, :qw]
                        nc.tensor.matmul(
                            pst,
                            kT[:, kvh, ti * P : (ti + 1) * P],
                            qT[:, h, q0 : q0 + qw],
                            start=True,
                            stop=True,
                        )
                        pTt = pTh[:, ti, q0 : q0 + qw]
                        nc.scalar.activation(
                            pTt,
                            pst,
                            mybir.ActivationFunctionType.Exp,
                            scale=inv_sqrt_dqk,
                            bias=bias_sb[:, ti : ti + 1],
                        )
                        for blk, off in mask_blocks[ti]:
                            bw = 2 * P if ti == 1 else P
                            nc.vector.tensor_tensor(
                                pTh[:, ti, off : off + bw],
                                pTh[:, ti, off : off + bw],
                                b_sb[:, blk : blk + bw // P, :],
                                op=mybir.AluOpType.mult,
                            )
                        # fold tile into the key-sum accumulator (DVE)
                        if ti == 1:
                            nc.vector.tensor_tensor(
                                acc[:],
                                pTh[:, 0, :],
                                pTh[:, 1, :],
                                op=mybir.AluOpType.add,
                            )
                        elif ti >= 2:
                            nc.vector.tensor_tensor(
                                acc[:, q0 : q0 + qw],
                                acc[:, q0 : q0 + qw],
                                pTt,
                                op=mybir.AluOpType.add,
                            )
                    return pTh, acc

                def emit_la(h, pTh, acc):
                    kvh = h // NREP
                    pso = psO.tile([P, SC], F32, tag="o", name="pso")
                    psl = psL.tile([1, SC], F32, tag="l", name="psl")
                    # 128-partition reduce of acc: single ones-column matmul
                    nc.tensor.matmul(
                        psl[:], ones_col[:], acc[:], start=True, stop=True
                    )
                    for ti in range(NT):
                        q0, qw = qr[ti]
                        nc.tensor.matmul(
                            pso[:, q0 : q0 + qw],
                            v_sb[:, ti, kvh * DV : (kvh + 1) * DV],
                            pTh[:, ti, q0 : q0 + qw],
                            start=(ti == 0),
                            stop=(ti == NT - 1),
                            skip_group_check=True,
                        )
                    tails[h] = (psl, pso)

                def tail_recip(h):
                    psl, _ = tails[h]
                    irl = pbs.tile([1, SC], F32, tag="irl", name="irl")
                    nc.vector.reciprocal_approx_fast(irl[:], psl[:])
                    lrow = pbs.tile([1, SC], FP16, tag="lrow", name="lrow")
                    nc.scalar.copy(lrow[:], irl[:])
                    return lrow

                def tail_apply(h, lrow):
                    _, pso = tails[h]
                    psbc = ps.tile([P, SC], F32, tag="big", name="psbc")
                    nc.tensor.matmul(
                        psbc[:], ones_row[:], lrow[:], start=True, stop=True
                    )
                    rlbc = pbs.tile([P, SC], FP16, tag="rlbc", name="rlbc")
                    nc.vector.tensor_copy(rlbc[:], psbc[:])
                    nc.vector.tensor_tensor(
                        outT[:, h, :], pso[:], rlbc[:], op=mybir.AluOpType.mult
                    )
                    tails[h] = None

                fetch_wq(0)
                emit_qproj(0)
                emit_qproj(1)
                for h in range(NH):
                    lr = None
                    if h > 0:
                        lr = tail_recip(h - 1)
                    pTh, acc = emit_scores(h)
                    if h + 2 < NH:
                        emit_qproj(h + 2)
                    if h > 0:
                        tail_apply(h - 1, lr)
                    emit_la(h, pTh, acc)
                tail_apply(NH - 1, tail_recip(NH - 1))

            # ---- Phase C: y = outT^T @ wo (stream wo in n-chunks) ----
            y_tiles = [
                pcy.tile([P, D], FP16, tag="y", name=f"y{i}")
                for i in range(NQTL)
            ]
            for ncl in range(4):
                wo_g = pc.tile([P, NH, SC], FP16, tag="wog", name="wo_g")
                nc.sync.dma_start(wo_g[:], wo_d[:, ncl])
                for qtl in range(NQTL):
                    psy = ps.tile([P, SC], F32, tag="big", name="psy")
                    for h in range(NH):
                        nc.tensor.matmul(
                            psy[:],
                            outT[:, h, qtl * P : (qtl + 1) * P],
                            wo_g[:, h, :],
                            start=(h == 0),
                            stop=(h == NH - 1),
                        )
                    nc.vector.tensor_copy(
                        y_tiles[qtl][:, ncl * SC : (ncl + 1) * SC], psy[:]
                    )
                    nc.sync.dma_start(
                        y_d[
                            qtl * P : (qtl + 1) * P,
                            ncl * SC : (ncl + 1) * SC,
                        ],
                        y_tiles[qtl][:, ncl * SC : (ncl + 1) * SC],
                    )
            pcy.release()
            pc.release()

    return nc


_PROGRAM = None


def _get_program():
    global _PROGRAM
    if _PROGRAM is None:
        _PROGRAM = build_program()
        _PROGRAM.finalize()
    return _PROGRAM


def _host_inputs(x, wq, wk, wv, wo):
    """Per-core input packing (all arrays contiguous, uniform shapes)."""
    F16 = np.float16
    x2 = np.asarray(x, np.float32).reshape(S, D)
    xT = np.ascontiguousarray(x2.T)  # [D, S]
    xr = xT.reshape(KO, P, S)  # [ko, p, s]

    # paired RoPE basis permutation within each head
    perm = np.concatenate([np.arange(0, DQK, 2), np.arange(1, DQK, 2)])
    wq_p = np.asarray(wq, np.float32).reshape(D, NH, DQK)[:, :, perm]
    wk_p = np.asarray(wk, np.float32).reshape(D, NKV, DQK)[:, :, perm]
    wv_r = np.asarray(wv, np.float32).reshape(D, NKV * DV)
    wo_r = np.asarray(wo, np.float32).reshape(NH, DV, D)

    # device layouts independent of core
    wq_dev = np.ascontiguousarray(
        wq_p.reshape(KO, P, NH, DQK)  # [ko, p, h, dqk]
        .reshape(KO, P, 4, 4 * DQK)  # group 4 heads
        .transpose(1, 2, 0, 3)  # [p, g, ko, 4*dqk]
    ).astype(F16)
    wk_dev = np.ascontiguousarray(
        wk_p.reshape(KO, P, NKV, DQK).transpose(1, 2, 0, 3)
    ).astype(F16)
    wv_dev = np.ascontiguousarray(
        wv_r.reshape(KO, P, NKV * DV).transpose(1, 0, 2)
    ).astype(F16)
    wo_dev = np.ascontiguousarray(
        wo_r.reshape(NH, DV, 4, SC).transpose(1, 2, 0, 3)  # [dv, ncl, h, sc]
    ).astype(F16)

    inv_freq = 1.0 / (THETA ** (np.arange(0, DQK, 2)[: DQK // 2] / DQK))

    in_maps = []
    for c in range(NC_):
        qlo = c * SC
        band_lo = qlo - WIN
        # packed key positions; garbage (pos<0) -> position 0, zero x
        pos = np.empty(NKC, np.int64)
        pos[:FRONT] = np.arange(FRONT)
        pos[FRONT:] = band_lo + np.arange(NKC - FRONT)
        valid = pos >= 0
        pos_c = np.where(valid, pos, 0)

        xp = xr[:, :, pos_c] * valid[None, None, :]  # [ko, p, nkc]
        if c == 0:
            # front tiles are dead on core 0 (band covers them); zero x so
            # their v projection is 0
            xp[:, :, :FRONT] = 0.0
        xp = np.ascontiguousarray(xp.transpose(1, 0, 2)).astype(F16)

        ang = np.outer(pos_c.astype(np.float64), inv_freq)  # (nkc, 64)
        cos_h = np.cos(ang).T.astype(np.float32)  # (64, nkc)
        sin_h = np.sin(ang).T.astype(np.float32)
        cos_p = np.ascontiguousarray(np.concatenate([cos_h, cos_h], 0)).astype(
            F16
        )
        sin_p = np.ascontiguousarray(np.concatenate([sin_h, sin_h], 0)).astype(
            F16
        )

        # Sparse 128x128 mask blocks (only blocks non-full on some core):
        #   blk 0: (front0, qtl0)  blk 1: (front1, qtl0)  blk 2: (front1, qtl1)
        #   blk 3+b: (band b, qtl b) for b<4 else (band b, qtl b-4)
        # Front tiles use only front_ok, band tiles only the band condition.
        r = np.arange(P)[:, None]
        B = np.zeros((P, 11, P), np.float32)
        blocks = [(0, 0), (1, 0), (1, 1)] + [
            (2 + b, b if b < 4 else b - 4) for b in range(8)
        ]
        for blk, (t, qtl) in enumerate(blocks):
            qpos = qlo + qtl * P + np.arange(P)[None, :]
            if t < 2:
                kpos = t * P + r
                allowed = (kpos < FRONT) & (kpos <= qpos - WIN)
            else:
                kpos = band_lo + (t - 2) * P + r
                allowed = (kpos >= 0) & (kpos <= qpos) & (kpos > qpos - WIN)
            B[:, blk, :] = allowed
        Bp = np.ascontiguousarray(B).astype(F16)

        # exp bias: kill core 0's dead tiles (front 0,1 + band 2..5 whose
        # packed positions are all negative) so they contribute ~0 to the
        # key-sum; all other (core, tile) pairs are fully live or handled
        # by the mask blocks.
        biasv = np.zeros((P, NT), np.float32)
        if c == 0:
            biasv[:, :6] = -60.0
        in_maps.append(
            {
                "xp": xp,
                "wq": wq_dev,
                "wk": wk_dev,
                "wv": wv_dev,
                "wo": wo_dev,
                "cosd": cos_p,
                "sind": sin_p,
                "bmask": Bp,
                "biasv": biasv,
            }
        )
    return in_maps


def kernel(x, wq, wk, wv, wo, _trace=False, _trace_kwargs=None):
    nc = _get_program()
    in_maps = _host_inputs(x, wq, wk, wv, wo)
    res = run_bass_kernel_spmd(
        nc, in_maps, list(range(NC_)), trace=_trace, **(_trace_kwargs or {})
    )
    y = np.concatenate(
        [np.asarray(r["y"], np.float32) for r in res.results], axis=0
    )
    out = y.reshape(1, S, D)
    if _trace:
        return out, res
    return out


# revision 13
# speedup vs baseline: 1.2574x; 1.0821x over previous
"""Sparse-attention (sliding window 512 + front 256) Trainium2 kernel.

Sequence-sharded across 8 NeuronCores: core c owns queries [512c, 512c+512)
and computes ALL 16 heads for them, including the full output projection, so
per-core output is a disjoint y slice [512, 2048] (no cross-core reduction).

Layout: scores are computed TRANSPOSED (sT[k, q] = kT_tile^T @ qT), so the
exp() output is already the [keys, q] rhs operand the attn@V matmul needs --
no PE transposes and no PSUM->SBUF staging copies.

v2 restructure (from trace analysis of v1 at 310us):
  - fp16 instead of bf16 everywhere (same PE rate, 4 extra mantissa bits).
  - q projection is interleaved INTO the attention head loop, so the PE
    stays saturated through the exp-heavy window (v1's phase B was
    Scalar-walled at ~105% busy while the PE idled).
  - softmax key-sum no longer runs on the PE (v1: 10 row-select matmuls per
    head, 57k cycles): the exp tiles accumulate on the DVE (fp16 2x mode)
    and a single ones-column matmul per head does the 128-partition reduce.
  - dead packed-key rows (core 0's padded band + covered front) are killed
    with a per-partition bias of -60 inside the exp activation itself
    (exp(s/sqrt(d) - 60) ~ 0), replacing the v1 row-select vector.

Per-core data variation (uniform single program):
  - packed key layout (NT=10 tiles): cols 0:256 front, cols 256:1280 band
    positions [qlo-512, qlo+512), zero-x for padding;
  - 11 sparse 128x128 {0,1} mask blocks multiplied into exp(scores);
  - biasv [128, NT]: -60 on core 0's dead tiles, 0 elsewhere.
"""
import math
import sys

import numpy as np

sys.path.insert(0, "/opt/trn_rl_repo")

import concourse.bass as bass
from concourse import bacc
import concourse.mybir as mybir
import concourse.tile as tile
from concourse.bass_utils import run_bass_kernel_spmd

# Problem constants (hardcoded per contract)
S = 4096
D = 2048
NH = 16
NKV = 4
NREP = NH // NKV
DQK = 128
DV = 128
WIN = 512
FRONT = 256
THETA = 10000.0
P = 128
NC_ = 8          # cores
SC = S // NC_    # 512 queries per core
KO = D // P      # 16 contraction chunks
NT = 10          # packed key tiles per core (2 front + 8 band)
NKC = NT * P     # 1280 packed key positions
NQTL = 4         # query tiles per core

F32 = mybir.dt.float32
FP16 = mybir.dt.float16


def build_program():
    nc = bacc.Bacc(None, target_bir_lowering=False)

    x_d = nc.dram_tensor("xp", [P, KO, NKC], FP16, kind="ExternalInput")
    wq_d = nc.dram_tensor("wq", [P, 4, KO, 4 * DQK], FP16, kind="ExternalInput")
    wk_d = nc.dram_tensor("wk", [P, NKV, KO, DQK], FP16, kind="ExternalInput")
    wv_d = nc.dram_tensor("wv", [P, KO, NKV * DV], FP16, kind="ExternalInput")
    wo_d = nc.dram_tensor("wo", [P, 4, NH, SC], FP16, kind="ExternalInput")
    cos_d = nc.dram_tensor("cosd", [P, NKC], FP16, kind="ExternalInput")
    sin_d = nc.dram_tensor("sind", [P, NKC], FP16, kind="ExternalInput")
    b_d = nc.dram_tensor("bmask", [P, 11, P], FP16, kind="ExternalInput")
    bias_d = nc.dram_tensor("biasv", [P, NT], F32, kind="ExternalInput")
    y_d = nc.dram_tensor("y", [SC, D], FP16, kind="ExternalOutput")

    inv_sqrt_dqk = 1.0 / math.sqrt(DQK)
    qc0 = NKC - SC  # first packed col of this core's own queries (768)

    # Per key tile t, the q columns that can attend it:
    #   front tiles (t=0,1): all 512;  band tile b: qtl in [b-4, b].
    qr = {0: (0, SC), 1: (0, SC)}
    for b in range(8):
        lo = max(0, b - 4) * P
        hi = (min(3, b) + 1) * P
        qr[2 + b] = (lo, hi - lo)
    # masked 128-col blocks per tile: (block index, absolute col offset)
    mask_blocks = {0: [(0, 0)], 1: [(1, 0)]}
    for b in range(8):
        off = (b if b < 4 else b - 4) * P
        mask_blocks[2 + b] = [(3 + b, off)]

    with tile.TileContext(nc) as tc:
        with (
            tc.tile_pool(name="persist", bufs=1) as persist,
            tc.tile_pool(name="ps", bufs=5, space="PSUM") as ps,
            tc.tile_pool(name="psO", bufs=2, space="PSUM") as psO,
            tc.tile_pool(name="psL", bufs=1, space="PSUM") as psL,
        ):
            # ---- persistent SBUF ----
            qT = persist.tile([P, NH, SC], FP16, tag="qT")
            kT = persist.tile([P, NKV, NKC], FP16, tag="kT")
            v_sb = persist.tile([P, NT, NKV * DV], FP16, tag="v")
            outT = persist.tile([P, NH, SC], FP16, tag="outT")
            b_sb = persist.tile([P, 11, P], FP16, tag="bm")
            bias_sb = persist.tile([P, NT], F32, tag="biasv")
            ones_row = persist.tile([1, P], FP16, tag="onesr")
            ones_col = persist.tile([P, 1], FP16, tag="onesc")
            # own-query x slice and cos/sin live through phase B (q proj is
            # interleaved there); the full packed x is phase-A scoped
            xq_sb = persist.tile([P, KO, SC], FP16, tag="xq")
            cos_sb = persist.tile([P, NKC], FP16, tag="cos")
            sin_sb = persist.tile([P, NKC], FP16, tag="sin")

            nc.vector.memset(ones_row[:], 1.0)
            nc.vector.memset(ones_col[:], 1.0)

            def rope(dst, psrc, cosap, sinap, pool, w):
                """dst(fp16) = RoPE(psrc) in the paired [re(64); im(64)] basis.

                sw = [-im; re]; dst = psrc*cos + sw*sin.
                """
                sw = pool.tile([P, w], F32, tag="sw")
                nc.scalar.mul(sw[0:64], psrc[64:128], -1.0)
                nc.scalar.copy(sw[64:128], psrc[0:64])
                trc = pool.tile([P, w], F32, tag="trc")
                nc.vector.tensor_tensor(
                    trc[:], psrc, cosap, op=mybir.AluOpType.mult
                )
                nc.vector.tensor_tensor(
                    sw[:], sw[:], sinap, op=mybir.AluOpType.mult
                )
                nc.vector.tensor_tensor(
                    dst, trc[:], sw[:], op=mybir.AluOpType.add
                )

            # ---- Phase A: k/v projections + k RoPE ----
            with tc.tile_pool(name="phA", bufs=1) as pa, tc.tile_pool(
                name="ropep", bufs=3
            ) as rp:
                x_sb = pa.tile([P, KO, NKC], FP16, tag="x")
                wk_sb = pa.tile([P, NKV, KO, DQK], FP16, tag="wk")
                wv_sb = pa.tile([P, KO, NKV * DV], FP16, tag="wv")

                nc.sync.dma_start(wk_sb[:, 0, 0:4], wk_d[:, 0, 0:4])
                nc.sync.dma_start(x_sb[:, 0:2, 0:512], x_d[:, 0:2, 0:512])
                nc.sync.dma_start(x_sb[:, 2:4, 0:512], x_d[:, 2:4, 0:512])
                nc.sync.dma_start(wk_sb[:, 0, 4:16], wk_d[:, 0, 4:16])
                for kg in range(4, KO, 4):
                    nc.sync.dma_start(
                        x_sb[:, kg : kg + 4, 0:512],
                        x_d[:, kg : kg + 4, 0:512],
                    )
                nc.sync.dma_start(cos_sb[:, 0:512], cos_d[:, 0:512])
                nc.sync.dma_start(sin_sb[:, 0:512], sin_d[:, 0:512])
                for _kvh in range(1, NKV):
                    nc.sync.dma_start(wk_sb[:, _kvh], wk_d[:, _kvh])
                nc.sync.dma_start(wv_sb[:], wv_d[:])
                for c0 in range(512, NKC, 512):
                    cw = min(512, NKC - c0)
                    nc.sync.dma_start(
                        x_sb[:, :, c0 : c0 + cw], x_d[:, :, c0 : c0 + cw]
                    )
                    nc.sync.dma_start(
                        cos_sb[:, c0 : c0 + cw], cos_d[:, c0 : c0 + cw]
                    )
                    nc.sync.dma_start(
                        sin_sb[:, c0 : c0 + cw], sin_d[:, c0 : c0 + cw]
                    )
                nc.sync.dma_start(b_sb[:], b_d[:])
                nc.sync.dma_start(bias_sb[:], bias_d[:])
                # own-query x slice for the interleaved q projection
                nc.sync.dma_start(xq_sb[:], x_d[:, :, qc0:NKC])

                # k+v projection, chunk-outer so compute starts on chunk 0
                for c0 in range(0, NKC, 512):
                    cw = min(512, NKC - c0)
                    for kvh in range(NKV):
                        psk = ps.tile([P, SC], F32, tag="big", name="psk")
                        psk = psk[:, :cw]
                        for ko in range(KO):
                            nc.tensor.matmul(
                                psk,
                                wk_sb[:, kvh, ko, :],
                                x_sb[:, ko, c0 : c0 + cw],
                                start=(ko == 0),
                                stop=(ko == KO - 1),
                            )
                        rope(
                            kT[:, kvh, c0 : c0 + cw],
                            psk,
                            cos_sb[:, c0 : c0 + cw],
                            sin_sb[:, c0 : c0 + cw],
                            rp,
                            cw,
                        )
                    # v projection (natural [keys, dv], all 4 kv heads)
                    for t in range(c0 // P, min(NT, (c0 + cw) // P)):
                        psv = ps.tile([P, SC], F32, tag="big", name="psv")
                        psv = psv[:, : NKV * DV]
                        for ko in range(KO):
                            nc.tensor.matmul(
                                psv,
                                x_sb[:, ko, t * P : (t + 1) * P],
                                wv_sb[:, ko, :],
                                start=(ko == 0),
                                stop=(ko == KO - 1),
                            )
                        nc.vector.tensor_copy(v_sb[:, t, :], psv)

            # ---- Phase B: q proj + RoPE + attention, per head, interleaved --
            # phase C pools opened now so wo prefetch overlaps phase B
            pc = tc.alloc_tile_pool(name="phC", bufs=2)
            pcy = tc.alloc_tile_pool(name="phCy", bufs=4)
            with (
                tc.tile_pool(name="phB", bufs=3) as pb,
                tc.tile_pool(name="phBs", bufs=4) as pbs,
                tc.tile_pool(name="wqs", bufs=2) as wqs,
                tc.tile_pool(name="ropeq", bufs=3) as rq,
            ):
                tails = [None] * NH  # (psl, pso) pending normalize
                wq_gs = [None] * (NH // 2)

                def fetch_wq(g):
                    # 2-head chunk: [P, KO, 2*DQK] slice of the 4-head group
                    gg, hh = divmod(2 * g, 4)
                    wq_g = wqs.tile([P, KO, 2 * DQK], FP16, tag="wqg")
                    nc.sync.dma_start(
                        wq_g[:], wq_d[:, gg, :, hh * DQK : (hh + 2) * DQK]
                    )
                    wq_gs[g] = wq_g

                def qproj_start(h):
                    g, hh = divmod(h, 2)
                    if hh == 0 and g + 1 < NH // 2:
                        fetch_wq(g + 1)
                    psq = ps.tile([P, SC], F32, tag="big", name="psq")
                    return [h, g, hh, psq, 0]

                def qproj_step(st, n):
                    _, g, hh, psq, ko = st
                    for k in range(ko, min(ko + n, KO)):
                        nc.tensor.matmul(
                            psq[:],
                            wq_gs[g][:, k, hh * DQK : (hh + 1) * DQK],
                            xq_sb[:, k, :],
                            start=(k == 0),
                            stop=(k == KO - 1),
                        )
                    st[4] = min(ko + n, KO)

                def qproj_finish(st):
                    h, _, _, psq, ko = st
                    assert ko == KO
                    rope(
                        qT[:, h, :],
                        psq[:],
                        cos_sb[:, qc0:NKC],
                        sin_sb[:, qc0:NKC],
                        rq,
                        SC,
                    )

                def emit_qproj(h):
                    st = qproj_start(h)
                    qproj_step(st, KO)
                    qproj_finish(st)

                def emit_scores(h, qst=None):
                    """scores + exp into one [P, NT, SC] tile; DVE-accumulate
                    the key-sum into acc; returns (pTh, acc)."""
                    kvh = h // NREP
                    pTh = pb.tile([P, NT, SC], FP16, tag="pT", name="pTh")
                    acc = pbs.tile([P, SC], FP16, tag="acc", name="acc")
                    for ti in range(NT):
                        q0, qw = qr[ti]
                        pst = ps.tile([P, SC], F32, tag="big", name="pst")
                        pst = pst[:, :qw]
                        nc.tensor.matmul(
                            pst,
                            kT[:, kvh, ti * P : (ti + 1) * P],
                            qT[:, h, q0 : q0 + qw],
                            start=True,
                            stop=True,
                        )
                        pTt = pTh[:, ti, q0 : q0 + qw]
                        nc.scalar.activation(
                            pTt,
                            pst,
                            mybir.ActivationFunctionType.Exp,
                            scale=inv_sqrt_dqk,
                            bias=bias_sb[:, ti : ti + 1],
                        )
                        for blk, off in mask_blocks[ti]:
                            bw = 2 * P if ti == 1 else P
                            nc.vector.tensor_tensor(
                                pTh[:, ti, off : off + bw],
                                pTh[:, ti, off : off + bw],
                                b_sb[:, blk : blk + bw // P, :],
                                op=mybir.AluOpType.mult,
                            )
                        # fold tile into the key-sum accumulator (DVE)
                        if ti == 1:
                            nc.vector.tensor_tensor(
                                acc[:],
                                pTh[:, 0, :],
                                pTh[:, 1, :],
                                op=mybir.AluOpType.add,
                            )
                        elif ti >= 2:
                            nc.vector.tensor_tensor(
                                acc[:, q0 : q0 + qw],
                                acc[:, q0 : q0 + qw],
                                pTt,
                                op=mybir.AluOpType.add,
                            )
                        # keep the PE fed with exp-independent work while the
                        # scalar engine drains score psum slots
                        if qst is not None:
                            qproj_step(qst, 2 if ti < 6 else 1)
                    return pTh, acc

                def emit_la(h, pTh, acc):
                    kvh = h // NREP
                    pso = psO.tile([P, SC], F32, tag="o", name="pso")
                    psl = psL.tile([1, SC], F32, tag="l", name="psl")
                    # 128-partition reduce of acc: single ones-column matmul
                    nc.tensor.matmul(
                        psl[:], ones_col[:], acc[:], start=True, stop=True
                    )
                    for ti in range(NT):
                        q0, qw = qr[ti]
                        nc.tensor.matmul(
                            pso[:, q0 : q0 + qw],
                            v_sb[:, ti, kvh * DV : (kvh + 1) * DV],
                            pTh[:, ti, q0 : q0 + qw],
                            start=(ti == 0),
                            stop=(ti == NT - 1),
                            skip_group_check=True,
                        )
                    tails[h] = (psl, pso)

                def tail_recip(h):
                    psl, _ = tails[h]
                    irl = pbs.tile([1, SC], F32, tag="irl", name="irl")
                    nc.vector.reciprocal_approx_fast(irl[:], psl[:])
                    lrow = pbs.tile([1, SC], FP16, tag="lrow", name="lrow")
                    nc.scalar.copy(lrow[:], irl[:])
                    return lrow

                def tail_apply(h, lrow):
                    _, pso = tails[h]
                    psbc = ps.tile([P, SC], F32, tag="big", name="psbc")
                    nc.tensor.matmul(
                        psbc[:], ones_row[:], lrow[:], start=True, stop=True
                    )
                    rlbc = pbs.tile([P, SC], FP16, tag="rlbc", name="rlbc")
                    nc.vector.tensor_copy(rlbc[:], psbc[:])
                    nc.vector.tensor_tensor(
                        outT[:, h, :], pso[:], rlbc[:], op=mybir.AluOpType.mult
                    )
                    tails[h] = None

                fetch_wq(0)
                emit_qproj(0)
                emit_qproj(1)
                lrows = {}
                for h in range(NH):
                    qst = qproj_start(h + 2) if h + 2 < NH else None
                    pTh, acc = emit_scores(h, qst)
                    if qst is not None:
                        qproj_finish(qst)
                    if h > 0:
                        tail_apply(h - 1, lrows.pop(h - 1))
                    emit_la(h, pTh, acc)
                    lrows[h] = tail_recip(h)
                tail_apply(NH - 1, lrows.pop(NH - 1))

            # ---- Phase C: y = outT^T @ wo (stream wo in n-chunks) ----
            y_tiles = [
                pcy.tile([P, D], FP16, tag="y", name=f"y{i}")
                for i in range(NQTL)
            ]
            for ncl in range(4):
                wo_g = pc.tile([P, NH, SC], FP16, tag="wog", name="wo_g")
                nc.sync.dma_start(wo_g[:], wo_d[:, ncl])
                for qtl in range(NQTL):
                    psy = ps.tile([P, SC], F32, tag="big", name="psy")
                    for h in range(NH):
                        nc.tensor.matmul(
                            psy[:],
                            outT[:, h, qtl * P : (qtl + 1) * P],
                            wo_g[:, h, :],
                            start=(h == 0),
                            stop=(h == NH - 1),
                        )
                    nc.vector.tensor_copy(
                        y_tiles[qtl][:, ncl * SC : (ncl + 1) * SC], psy[:]
                    )
                    nc.sync.dma_start(
                        y_d[
                            qtl * P : (qtl + 1) * P,
                            ncl * SC : (ncl + 1) * SC,
                        ],
                        y_tiles[qtl][:, ncl * SC : (ncl + 1) * SC],
                    )
            pcy.release()
            pc.release()

    return nc


_PROGRAM = None


def _get_program():
    global _PROGRAM
    if _PROGRAM is None:
        _PROGRAM = build_program()
        _PROGRAM.finalize()
    return _PROGRAM


def _host_inputs(x, wq, wk, wv, wo):
    """Per-core input packing (all arrays contiguous, uniform shapes)."""
    F16 = np.float16
    x2 = np.asarray(x, np.float32).reshape(S, D)
    xT = np.ascontiguousarray(x2.T)  # [D, S]
    xr = xT.reshape(KO, P, S)  # [ko, p, s]

    # paired RoPE basis permutation within each head
    perm = np.concatenate([np.arange(0, DQK, 2), np.arange(1, DQK, 2)])
    wq_p = np.asarray(wq, np.float32).reshape(D, NH, DQK)[:, :, perm]
    wk_p = np.asarray(wk, np.float32).reshape(D, NKV, DQK)[:, :, perm]
    wv_r = np.asarray(wv, np.float32).reshape(D, NKV * DV)
    wo_r = np.asarray(wo, np.float32).reshape(NH, DV, D)

    # device layouts independent of core
    wq_dev = np.ascontiguousarray(
        wq_p.reshape(KO, P, NH, DQK)  # [ko, p, h, dqk]
        .reshape(KO, P, 4, 4 * DQK)  # group 4 heads
        .transpose(1, 2, 0, 3)  # [p, g, ko, 4*dqk]
    ).astype(F16)
    wk_dev = np.ascontiguousarray(
        wk_p.reshape(KO, P, NKV, DQK).transpose(1, 2, 0, 3)
    ).astype(F16)
    wv_dev = np.ascontiguousarray(
        wv_r.reshape(KO, P, NKV * DV).transpose(1, 0, 2)
    ).astype(F16)
    wo_dev = np.ascontiguousarray(
        wo_r.reshape(NH, DV, 4, SC).transpose(1, 2, 0, 3)  # [dv, ncl, h, sc]
    ).astype(F16)

    inv_freq = 1.0 / (THETA ** (np.arange(0, DQK, 2)[: DQK // 2] / DQK))

    in_maps = []
    for c in range(NC_):
        qlo = c * SC
        band_lo = qlo - WIN
        # packed key positions; garbage (pos<0) -> position 0, zero x
        pos = np.empty(NKC, np.int64)
        pos[:FRONT] = np.arange(FRONT)
        pos[FRONT:] = band_lo + np.arange(NKC - FRONT)
        valid = pos >= 0
        pos_c = np.where(valid, pos, 0)

        xp = xr[:, :, pos_c] * valid[None, None, :]  # [ko, p, nkc]
        if c == 0:
            # front tiles are dead on core 0 (band covers them); zero x so
            # their v projection is 0
            xp[:, :, :FRONT] = 0.0
        xp = np.ascontiguousarray(xp.transpose(1, 0, 2)).astype(F16)

        ang = np.outer(pos_c.astype(np.float64), inv_freq)  # (nkc, 64)
        cos_h = np.cos(ang).T.astype(np.float32)  # (64, nkc)
        sin_h = np.sin(ang).T.astype(np.float32)
        cos_p = np.ascontiguousarray(np.concatenate([cos_h, cos_h], 0)).astype(
            F16
        )
        sin_p = np.ascontiguousarray(np.concatenate([sin_h, sin_h], 0)).astype(
            F16
        )

        # Sparse 128x128 mask blocks (only blocks non-full on some core):
        #   blk 0: (front0, qtl0)  blk 1: (front1, qtl0)  blk 2: (front1, qtl1)
        #   blk 3+b: (band b, qtl b) for b<4 else (band b, qtl b-4)
        # Front tiles use only front_ok, band tiles only the band condition.
        r = np.arange(P)[:, None]
        B = np.zeros((P, 11, P), np.float32)
        blocks = [(0, 0), (1, 0), (1, 1)] + [
            (2 + b, b if b < 4 else b - 4) for b in range(8)
        ]
        for blk, (t, qtl) in enumerate(blocks):
            qpos = qlo + qtl * P + np.arange(P)[None, :]
            if t < 2:
                kpos = t * P + r
                allowed = (kpos < FRONT) & (kpos <= qpos - WIN)
            else:
                kpos = band_lo + (t - 2) * P + r
                allowed = (kpos >= 0) & (kpos <= qpos) & (kpos > qpos - WIN)
            B[:, blk, :] = allowed
        Bp = np.ascontiguousarray(B).astype(F16)

        # exp bias: kill core 0's dead tiles (front 0,1 + band 2..5 whose
        # packed positions are all negative) so they contribute ~0 to the
        # key-sum; all other (core, tile) pairs are fully live or handled
        # by the mask blocks.
        biasv = np.zeros((P, NT), np.float32)
        if c == 0:
            biasv[:, :6] = -60.0
        in_maps.append(
            {
                "xp": xp,
                "wq": wq_dev,
                "wk": wk_dev,
                "wv": wv_dev,
                "wo": wo_dev,
                "cosd": cos_p,
                "sind": sin_p,
                "bmask": Bp,
                "biasv": biasv,
            }
        )
    return in_maps


def kernel(x, wq, wk, wv, wo, _trace=False, _trace_kwargs=None):
    nc = _get_program()
    in_maps = _host_inputs(x, wq, wk, wv, wo)
    res = run_bass_kernel_spmd(
        nc, in_maps, list(range(NC_)), trace=_trace, **(_trace_kwargs or {})
    )
    y = np.concatenate(
        [np.asarray(r["y"], np.float32) for r in res.results], axis=0
    )
    out = y.reshape(1, S, D)
    if _trace:
        return out, res
    return out


# revision 15
# speedup vs baseline: 1.2836x; 1.0208x over previous
"""Sparse-attention (sliding window 512 + front 256) Trainium2 kernel.

Sequence-sharded across 8 NeuronCores: core c owns queries [512c, 512c+512)
and computes ALL 16 heads for them, including the full output projection, so
per-core output is a disjoint y slice [512, 2048] (no cross-core reduction).

Layout: scores are computed TRANSPOSED (sT[k, q] = kT_tile^T @ qT), so the
exp() output is already the [keys, q] rhs operand the attn@V matmul needs --
no PE transposes and no PSUM->SBUF staging copies.

v2 restructure (from trace analysis of v1 at 310us):
  - fp16 instead of bf16 everywhere (same PE rate, 4 extra mantissa bits).
  - q projection is interleaved INTO the attention head loop, so the PE
    stays saturated through the exp-heavy window (v1's phase B was
    Scalar-walled at ~105% busy while the PE idled).
  - softmax key-sum no longer runs on the PE (v1: 10 row-select matmuls per
    head, 57k cycles): the exp tiles accumulate on the DVE (fp16 2x mode)
    and a single ones-column matmul per head does the 128-partition reduce.
  - dead packed-key rows (core 0's padded band + covered front) are killed
    with a per-partition bias of -60 inside the exp activation itself
    (exp(s/sqrt(d) - 60) ~ 0), replacing the v1 row-select vector.

Per-core data variation (uniform single program):
  - packed key layout (NT=10 tiles): cols 0:256 front, cols 256:1280 band
    positions [qlo-512, qlo+512), zero-x for padding;
  - 11 sparse 128x128 {0,1} mask blocks multiplied into exp(scores);
  - biasv [128, NT]: -60 on core 0's dead tiles, 0 elsewhere.
"""
import math
import sys

import numpy as np

sys.path.insert(0, "/opt/trn_rl_repo")

import concourse.bass as bass
from concourse import bacc
import concourse.mybir as mybir
import concourse.tile as tile
from concourse.bass_utils import run_bass_kernel_spmd

# Problem constants (hardcoded per contract)
S = 4096
D = 2048
NH = 16
NKV = 4
NREP = NH // NKV
DQK = 128
DV = 128
WIN = 512
FRONT = 256
THETA = 10000.0
P = 128
NC_ = 8          # cores
SC = S // NC_    # 512 queries per core
KO = D // P      # 16 contraction chunks
NT = 10          # packed key tiles per core (2 front + 8 band)
NKC = NT * P     # 1280 packed key positions
NQTL = 4         # query tiles per core

F32 = mybir.dt.float32
FP16 = mybir.dt.float16


def build_program():
    nc = bacc.Bacc(None, target_bir_lowering=False)

    x_d = nc.dram_tensor("xp", [P, KO, NKC], FP16, kind="ExternalInput")
    wq_d = nc.dram_tensor("wq", [P, 4, KO, 4 * DQK], FP16, kind="ExternalInput")
    wk_d = nc.dram_tensor("wk", [P, NKV, KO, DQK], FP16, kind="ExternalInput")
    wv_d = nc.dram_tensor("wv", [P, KO, NKV * DV], FP16, kind="ExternalInput")
    wo_d = nc.dram_tensor("wo", [P, 4, NH, SC], FP16, kind="ExternalInput")
    cos_d = nc.dram_tensor("cosd", [P, NKC], FP16, kind="ExternalInput")
    sin_d = nc.dram_tensor("sind", [P, NKC], FP16, kind="ExternalInput")
    b_d = nc.dram_tensor("bmask", [P, 11, P], FP16, kind="ExternalInput")
    bias_d = nc.dram_tensor("biasv", [P, NT], F32, kind="ExternalInput")
    y_d = nc.dram_tensor("y", [SC, D], FP16, kind="ExternalOutput")

    inv_sqrt_dqk = 1.0 / math.sqrt(DQK)
    qc0 = NKC - SC  # first packed col of this core's own queries (768)

    # Per key tile t, the q columns that can attend it:
    #   front tiles (t=0,1): all 512;  band tile b: qtl in [b-4, b].
    qr = {0: (0, SC), 1: (0, SC)}
    for b in range(8):
        lo = max(0, b - 4) * P
        hi = (min(3, b) + 1) * P
        qr[2 + b] = (lo, hi - lo)
    # masked 128-col blocks per tile: (block index, absolute col offset)
    mask_blocks = {0: [(0, 0)], 1: [(1, 0)]}
    for b in range(8):
        off = (b if b < 4 else b - 4) * P
        mask_blocks[2 + b] = [(3 + b, off)]

    with tile.TileContext(nc) as tc:
        with (
            tc.tile_pool(name="persist", bufs=1) as persist,
            tc.tile_pool(name="ps", bufs=4, space="PSUM") as ps,
            tc.tile_pool(name="psO", bufs=3, space="PSUM") as psO,
            tc.tile_pool(name="psL", bufs=1, space="PSUM") as psL,
        ):
            # ---- persistent SBUF ----
            qT = persist.tile([P, NH, SC], FP16, tag="qT")
            kT = persist.tile([P, NKV, NKC], FP16, tag="kT")
            v_sb = persist.tile([P, NT, NKV * DV], FP16, tag="v")
            outT = persist.tile([P, NH, SC], FP16, tag="outT")
            b_sb = persist.tile([P, 11, P], FP16, tag="bm")
            bias_sb = persist.tile([P, NT], F32, tag="biasv")
            ones_row = persist.tile([1, P], FP16, tag="onesr")
            ones_col = persist.tile([P, 1], FP16, tag="onesc")
            # own-query x slice and cos/sin live through phase B (q proj is
            # interleaved there); the full packed x is phase-A scoped
            xq_sb = persist.tile([P, KO, SC], FP16, tag="xq")
            cos_sb = persist.tile([P, NKC], FP16, tag="cos")
            sin_sb = persist.tile([P, NKC], FP16, tag="sin")

            nc.vector.memset(ones_row[:], 1.0)
            nc.vector.memset(ones_col[:], 1.0)

            def rope(dst, psrc, cosap, sinap, pool, w):
                """dst(fp16) = RoPE(psrc) in the paired [re(64); im(64)] basis.

                sw = [-im; re]; dst = psrc*cos + sw*sin.
                """
                sw = pool.tile([P, w], F32, tag="sw")
                nc.scalar.mul(sw[0:64], psrc[64:128], -1.0)
                nc.scalar.copy(sw[64:128], psrc[0:64])
                trc = pool.tile([P, w], F32, tag="trc")
                nc.vector.tensor_tensor(
                    trc[:], psrc, cosap, op=mybir.AluOpType.mult
                )
                nc.vector.tensor_tensor(
                    sw[:], sw[:], sinap, op=mybir.AluOpType.mult
                )
                nc.vector.tensor_tensor(
                    dst, trc[:], sw[:], op=mybir.AluOpType.add
                )

            # ---- Phase A: k/v projections + k RoPE ----
            with tc.tile_pool(name="phA", bufs=1) as pa, tc.tile_pool(
                name="ropep", bufs=3
            ) as rp:
                x_sb = pa.tile([P, KO, NKC], FP16, tag="x")
                wk_sb = pa.tile([P, NKV, KO, DQK], FP16, tag="wk")
                wv_sb = pa.tile([P, KO, NKV * DV], FP16, tag="wv")

                nc.sync.dma_start(wk_sb[:, 0, 0:4], wk_d[:, 0, 0:4])
                nc.sync.dma_start(x_sb[:, 0:2, 0:512], x_d[:, 0:2, 0:512])
                nc.sync.dma_start(x_sb[:, 2:4, 0:512], x_d[:, 2:4, 0:512])
                nc.sync.dma_start(wk_sb[:, 0, 4:16], wk_d[:, 0, 4:16])
                for kg in range(4, KO, 4):
                    nc.sync.dma_start(
                        x_sb[:, kg : kg + 4, 0:512],
                        x_d[:, kg : kg + 4, 0:512],
                    )
                nc.sync.dma_start(cos_sb[:, 0:512], cos_d[:, 0:512])
                nc.sync.dma_start(sin_sb[:, 0:512], sin_d[:, 0:512])
                for _kvh in range(1, NKV):
                    nc.sync.dma_start(wk_sb[:, _kvh], wk_d[:, _kvh])
                nc.sync.dma_start(wv_sb[:], wv_d[:])
                for c0 in range(512, NKC, 512):
                    cw = min(512, NKC - c0)
                    nc.sync.dma_start(
                        x_sb[:, :, c0 : c0 + cw], x_d[:, :, c0 : c0 + cw]
                    )
                    nc.sync.dma_start(
                        cos_sb[:, c0 : c0 + cw], cos_d[:, c0 : c0 + cw]
                    )
                    nc.sync.dma_start(
                        sin_sb[:, c0 : c0 + cw], sin_d[:, c0 : c0 + cw]
                    )
                nc.sync.dma_start(b_sb[:], b_d[:])
                nc.sync.dma_start(bias_sb[:], bias_d[:])
                # own-query x slice for the interleaved q projection
                nc.sync.dma_start(xq_sb[:], x_d[:, :, qc0:NKC])

                # k+v projection, chunk-outer so compute starts on chunk 0
                for c0 in range(0, NKC, 512):
                    cw = min(512, NKC - c0)
                    for kvh in range(NKV):
                        psk = ps.tile([P, SC], F32, tag="big", name="psk")
                        psk = psk[:, :cw]
                        for ko in range(KO):
                            nc.tensor.matmul(
                                psk,
                                wk_sb[:, kvh, ko, :],
                                x_sb[:, ko, c0 : c0 + cw],
                                start=(ko == 0),
                                stop=(ko == KO - 1),
                            )
                        rope(
                            kT[:, kvh, c0 : c0 + cw],
                            psk,
                            cos_sb[:, c0 : c0 + cw],
                            sin_sb[:, c0 : c0 + cw],
                            rp,
                            cw,
                        )
                    # v projection (natural [keys, dv], all 4 kv heads)
                    for t in range(c0 // P, min(NT, (c0 + cw) // P)):
                        psv = ps.tile([P, SC], F32, tag="big", name="psv")
                        psv = psv[:, : NKV * DV]
                        for ko in range(KO):
                            nc.tensor.matmul(
                                psv,
                                x_sb[:, ko, t * P : (t + 1) * P],
                                wv_sb[:, ko, :],
                                start=(ko == 0),
                                stop=(ko == KO - 1),
                            )
                        nc.vector.tensor_copy(v_sb[:, t, :], psv)

            # ---- Phase B: q proj + RoPE + attention, per head, interleaved --
            # phase C pools opened now so wo prefetch overlaps phase B
            pc = tc.alloc_tile_pool(name="phC", bufs=2)
            pcy = tc.alloc_tile_pool(name="phCy", bufs=4)
            with (
                tc.tile_pool(name="phB", bufs=3) as pb,
                tc.tile_pool(name="phBs", bufs=4) as pbs,
                tc.tile_pool(name="wqs", bufs=2) as wqs,
                tc.tile_pool(name="ropeq", bufs=3) as rq,
            ):
                tails = [None] * NH  # (psl, pso) pending normalize
                wq_gs = [None] * (NH // 2)

                def fetch_wq(g):
                    # 2-head chunk: [P, KO, 2*DQK] slice of the 4-head group
                    gg, hh = divmod(2 * g, 4)
                    wq_g = wqs.tile([P, KO, 2 * DQK], FP16, tag="wqg")
                    nc.sync.dma_start(
                        wq_g[:], wq_d[:, gg, :, hh * DQK : (hh + 2) * DQK]
                    )
                    wq_gs[g] = wq_g

                def qproj_start(h):
                    g, hh = divmod(h, 2)
                    if hh == 0 and g + 1 < NH // 2:
                        fetch_wq(g + 1)
                    psq = ps.tile([P, SC], F32, tag="big", name="psq")
                    return [h, g, hh, psq, 0]

                def qproj_step(st, n):
                    _, g, hh, psq, ko = st
                    for k in range(ko, min(ko + n, KO)):
                        nc.tensor.matmul(
                            psq[:],
                            wq_gs[g][:, k, hh * DQK : (hh + 1) * DQK],
                            xq_sb[:, k, :],
                            start=(k == 0),
                            stop=(k == KO - 1),
                        )
                    st[4] = min(ko + n, KO)

                def qproj_finish(st):
                    h, _, _, psq, ko = st
                    assert ko == KO
                    rope(
                        qT[:, h, :],
                        psq[:],
                        cos_sb[:, qc0:NKC],
                        sin_sb[:, qc0:NKC],
                        rq,
                        SC,
                    )

                def emit_qproj(h):
                    st = qproj_start(h)
                    qproj_step(st, KO)
                    qproj_finish(st)

                def emit_scores(h, qst=None):
                    """scores + exp into one [P, NT, SC] tile; DVE-accumulate
                    the key-sum into acc; returns (pTh, acc)."""
                    kvh = h // NREP
                    pTh = pb.tile([P, NT, SC], FP16, tag="pT", name="pTh")
                    acc = pbs.tile([P, SC], FP16, tag="acc", name="acc")
                    for ti in range(NT):
                        q0, qw = qr[ti]
                        pst = ps.tile([P, SC], F32, tag="big", name="pst")
                        pst = pst[:, :qw]
                        nc.tensor.matmul(
                            pst,
                            kT[:, kvh, ti * P : (ti + 1) * P],
                            qT[:, h, q0 : q0 + qw],
                            start=True,
                            stop=True,
                        )
                        pTt = pTh[:, ti, q0 : q0 + qw]
                        nc.scalar.activation(
                            pTt,
                            pst,
                            mybir.ActivationFunctionType.Exp,
                            scale=inv_sqrt_dqk,
                            bias=bias_sb[:, ti : ti + 1],
                        )
                        for blk, off in mask_blocks[ti]:
                            bw = 2 * P if ti == 1 else P
                            nc.vector.tensor_tensor(
                                pTh[:, ti, off : off + bw],
                                pTh[:, ti, off : off + bw],
                                b_sb[:, blk : blk + bw // P, :],
                                op=mybir.AluOpType.mult,
                            )
                        # fold tile into the key-sum accumulator (DVE)
                        if ti == 1:
                            nc.vector.tensor_tensor(
                                acc[:],
                                pTh[:, 0, :],
                                pTh[:, 1, :],
                                op=mybir.AluOpType.add,
                            )
                        elif ti >= 2:
                            nc.vector.tensor_tensor(
                                acc[:, q0 : q0 + qw],
                                acc[:, q0 : q0 + qw],
                                pTt,
                                op=mybir.AluOpType.add,
                            )
                        # keep the PE fed with exp-independent work while the
                        # scalar engine drains score psum slots
                        if qst is not None:
                            qproj_step(qst, 2 if ti < 6 else 1)
                    return pTh, acc

                def emit_la(h, pTh, acc):
                    kvh = h // NREP
                    pso = psO.tile([P, SC], F32, tag="o", name="pso")
                    psl = psL.tile([1, SC], F32, tag="l", name="psl")
                    # 128-partition reduce of acc: single ones-column matmul
                    nc.tensor.matmul(
                        psl[:], ones_col[:], acc[:], start=True, stop=True
                    )
                    for ti in range(NT):
                        q0, qw = qr[ti]
                        nc.tensor.matmul(
                            pso[:, q0 : q0 + qw],
                            v_sb[:, ti, kvh * DV : (kvh + 1) * DV],
                            pTh[:, ti, q0 : q0 + qw],
                            start=(ti == 0),
                            stop=(ti == NT - 1),
                            skip_group_check=True,
                        )
                    tails[h] = (psl, pso)

                def tail_recip(h):
                    psl, _ = tails[h]
                    irl = pbs.tile([1, SC], F32, tag="irl", name="irl")
                    nc.vector.reciprocal_approx_fast(irl[:], psl[:])
                    lrow = pbs.tile([1, SC], FP16, tag="lrow", name="lrow")
                    nc.scalar.copy(lrow[:], irl[:])
                    return lrow

                def tail_apply(h, lrow):
                    _, pso = tails[h]
                    psbc = ps.tile([P, SC], F32, tag="big", name="psbc")
                    nc.tensor.matmul(
                        psbc[:], ones_row[:], lrow[:], start=True, stop=True
                    )
                    rlbc = pbs.tile([P, SC], FP16, tag="rlbc", name="rlbc")
                    nc.scalar.copy(rlbc[:], psbc[:])
                    nc.vector.tensor_tensor(
                        outT[:, h, :], pso[:], rlbc[:], op=mybir.AluOpType.mult
                    )
                    tails[h] = None

                fetch_wq(0)
                emit_qproj(0)
                emit_qproj(1)
                lrows = {}
                for h in range(NH):
                    qst = qproj_start(h + 2) if h + 2 < NH else None
                    pTh, acc = emit_scores(h, qst)
                    if qst is not None:
                        qproj_finish(qst)
                    if h > 0:
                        tail_apply(h - 1, lrows.pop(h - 1))
                    emit_la(h, pTh, acc)
                    lrows[h] = tail_recip(h)
                tail_apply(NH - 1, lrows.pop(NH - 1))

            # ---- Phase C: y = outT^T @ wo (stream wo in n-chunks) ----
            y_tiles = [
                pcy.tile([P, D], FP16, tag="y", name=f"y{i}")
                for i in range(NQTL)
            ]
            for ncl in range(4):
                wo_g = pc.tile([P, NH, SC], FP16, tag="wog", name="wo_g")
                nc.sync.dma_start(wo_g[:], wo_d[:, ncl])
                for qtl in range(NQTL):
                    psy = ps.tile([P, SC], F32, tag="big", name="psy")
                    for h in range(NH):
                        nc.tensor.matmul(
                            psy[:],
                            outT[:, h, qtl * P : (qtl + 1) * P],
                            wo_g[:, h, :],
                            start=(h == 0),
                            stop=(h == NH - 1),
                        )
                    nc.vector.tensor_copy(
                        y_tiles[qtl][:, ncl * SC : (ncl + 1) * SC], psy[:]
                    )
                    nc.sync.dma_start(
                        y_d[
                            qtl * P : (qtl + 1) * P,
                            ncl * SC : (ncl + 1) * SC,
                        ],
                        y_tiles[qtl][:, ncl * SC : (ncl + 1) * SC],
                    )
            pcy.release()
            pc.release()

    return nc


_PROGRAM = None


def _get_program():
    global _PROGRAM
    if _PROGRAM is None:
        _PROGRAM = build_program()
        _PROGRAM.finalize()
    return _PROGRAM


def _host_inputs(x, wq, wk, wv, wo):
    """Per-core input packing (all arrays contiguous, uniform shapes)."""
    F16 = np.float16
    x2 = np.asarray(x, np.float32).reshape(S, D)
    xT = np.ascontiguousarray(x2.T)  # [D, S]
    xr = xT.reshape(KO, P, S)  # [ko, p, s]

    # paired RoPE basis permutation within each head
    perm = np.concatenate([np.arange(0, DQK, 2), np.arange(1, DQK, 2)])
    wq_p = np.asarray(wq, np.float32).reshape(D, NH, DQK)[:, :, perm]
    wk_p = np.asarray(wk, np.float32).reshape(D, NKV, DQK)[:, :, perm]
    wv_r = np.asarray(wv, np.float32).reshape(D, NKV * DV)
    wo_r = np.asarray(wo, np.float32).reshape(NH, DV, D)

    # device layouts independent of core
    wq_dev = np.ascontiguousarray(
        wq_p.reshape(KO, P, NH, DQK)  # [ko, p, h, dqk]
        .reshape(KO, P, 4, 4 * DQK)  # group 4 heads
        .transpose(1, 2, 0, 3)  # [p, g, ko, 4*dqk]
    ).astype(F16)
    wk_dev = np.ascontiguousarray(
        wk_p.reshape(KO, P, NKV, DQK).transpose(1, 2, 0, 3)
    ).astype(F16)
    wv_dev = np.ascontiguousarray(
        wv_r.reshape(KO, P, NKV * DV).transpose(1, 0, 2)
    ).astype(F16)
    wo_dev = np.ascontiguousarray(
        wo_r.reshape(NH, DV, 4, SC).transpose(1, 2, 0, 3)  # [dv, ncl, h, sc]
    ).astype(F16)

    inv_freq = 1.0 / (THETA ** (np.arange(0, DQK, 2)[: DQK // 2] / DQK))

    in_maps = []
    for c in range(NC_):
        qlo = c * SC
        band_lo = qlo - WIN
        # packed key positions; garbage (pos<0) -> position 0, zero x
        pos = np.empty(NKC, np.int64)
        pos[:FRONT] = np.arange(FRONT)
        pos[FRONT:] = band_lo + np.arange(NKC - FRONT)
        valid = pos >= 0
        pos_c = np.where(valid, pos, 0)

        xp = xr[:, :, pos_c] * valid[None, None, :]  # [ko, p, nkc]
        if c == 0:
            # front tiles are dead on core 0 (band covers them); zero x so
            # their v projection is 0
            xp[:, :, :FRONT] = 0.0
        xp = np.ascontiguousarray(xp.transpose(1, 0, 2)).astype(F16)

        ang = np.outer(pos_c.astype(np.float64), inv_freq)  # (nkc, 64)
        cos_h = np.cos(ang).T.astype(np.float32)  # (64, nkc)
        sin_h = np.sin(ang).T.astype(np.float32)
        cos_p = np.ascontiguousarray(np.concatenate([cos_h, cos_h], 0)).astype(
            F16
        )
        sin_p = np.ascontiguousarray(np.concatenate([sin_h, sin_h], 0)).astype(
            F16
        )

        # Sparse 128x128 mask blocks (only blocks non-full on some core):
        #   blk 0: (front0, qtl0)  blk 1: (front1, qtl0)  blk 2: (front1, qtl1)
        #   blk 3+b: (band b, qtl b) for b<4 else (band b, qtl b-4)
        # Front tiles use only front_ok, band tiles only the band condition.
        r = np.arange(P)[:, None]
        B = np.zeros((P, 11, P), np.float32)
        blocks = [(0, 0), (1, 0), (1, 1)] + [
            (2 + b, b if b < 4 else b - 4) for b in range(8)
        ]
        for blk, (t, qtl) in enumerate(blocks):
            qpos = qlo + qtl * P + np.arange(P)[None, :]
            if t < 2:
                kpos = t * P + r
                allowed = (kpos < FRONT) & (kpos <= qpos - WIN)
            else:
                kpos = band_lo + (t - 2) * P + r
                allowed = (kpos >= 0) & (kpos <= qpos) & (kpos > qpos - WIN)
            B[:, blk, :] = allowed
        Bp = np.ascontiguousarray(B).astype(F16)

        # exp bias: kill core 0's dead tiles (front 0,1 + band 2..5 whose
        # packed positions are all negative) so they contribute ~0 to the
        # key-sum; all other (core, tile) pairs are fully live or handled
        # by the mask blocks.
        biasv = np.zeros((P, NT), np.float32)
        if c == 0:
            biasv[:, :6] = -60.0
        in_maps.append(
            {
                "xp": xp,
                "wq": wq_dev,
                "wk": wk_dev,
                "wv": wv_dev,
                "wo": wo_dev,
                "cosd": cos_p,
                "sind": sin_p,
                "bmask": Bp,
                "biasv": biasv,
            }
        )
    return in_maps


def kernel(x, wq, wk, wv, wo, _trace=False, _trace_kwargs=None):
    nc = _get_program()
    in_maps = _host_inputs(x, wq, wk, wv, wo)
    res = run_bass_kernel_spmd(
        nc, in_maps, list(range(NC_)), trace=_trace, **(_trace_kwargs or {})
    )
    y = np.concatenate(
        [np.asarray(r["y"], np.float32) for r in res.results], axis=0
    )
    out = y.reshape(1, S, D)
    if _trace:
        return out, res
    return out
